# revision 2
# baseline (speedup 1.0000x reference)
"""ClassAttention kernel for 8 Trainium2 NeuronCores.

Problem: B=32, N=4096, C=768, H=12 single-CLS-query attention:
    q  = (x[:, :1] @ Wq) * scale          # [B,1,C] -> per-head q_h [64]
    kv = x @ Wkv                          # [B,N,2C]
    cls = softmax(q k^T) v                # per head, single query
    out = cls @ Wp + bp                   # [B,1,768]

Key restructuring: with a single query per (batch, head) the k/v projections
factor through the attention algebraically:
    scores_h,n = q_h . (x_n Wk_h) = (Wk_h q_h) . x_n        =: qt_h . x_n
    out_h      = (sum_n p_n (x_n Wv_h)) / den = ((sum_n p_n x_n) Wv_h) / den
so the kernel never computes the [N, 2C] kv projection at all.  Per token we
only need scores (rank-12 product against x^T) and a 12-row weighted sum of x
-- ~60x fewer FLOPs than the naive form; the kernel is memory-bound streaming
x once from HBM.  exp() runs without max-subtraction: scores are ~N(0,1)
(|s|max ~ 5 over the whole input set), so fp32 exp is safe.

Sharding: data-parallel over B: 8 cores x 4 batches.  No collectives.
"""

import sys

for _p in ("/opt/trn_rl_repo",):
    if _p not in sys.path:
        sys.path.insert(0, _p)

import numpy as np

import concourse.bass as bass
import concourse.mybir as mybir
import concourse.tile as tile
from concourse import bacc
from concourse.bass_utils import run_bass_kernel_spmd
from concourse.masks import make_identity

# Problem constants (hardcoded per the harness contract)
B, N, C, H = 32, 4096, 768, 12
D = C // H
SCALE = float(D) ** -0.5
NCORES = 8
BL = B // NCORES          # batches per core
P = 128
NCH = C // P              # 6 C-chunks of 128
ST = 512                  # tokens per supertile
S = ST // P               # token groups per supertile (tokens interleaved p*S+s)
NST = N // ST             # supertiles per batch

F32 = mybir.dt.float32
CD = mybir.dt.bfloat16    # compute dtype for matmul operands

HALF = 384                # psum-bank-sized half of C for [12, C] accumulators


def build():
    nc = bacc.Bacc("TRN2", target_bir_lowering=False, num_devices=NCORES)

    x_t = nc.dram_tensor("x", [BL, N, C], F32, kind="ExternalInput")
    wq_t = nc.dram_tensor("Wq", [C, C], F32, kind="ExternalInput")
    wkv_t = nc.dram_tensor("Wkv", [C, 2 * C], F32, kind="ExternalInput")
    wp_t = nc.dram_tensor("Wp", [C, C], F32, kind="ExternalInput")
    bp_t = nc.dram_tensor("bp", [C], F32, kind="ExternalInput")
    out_t = nc.dram_tensor("out", [BL, 1, C], F32, kind="ExternalOutput")

    with tile.TileContext(nc) as tc:
        _build_tiles(nc, tc, x_t, wq_t, wkv_t, wp_t, bp_t, out_t)
    nc.finalize()
    return nc


def _build_tiles(nc, tc, x_t, wq_t, wkv_t, wp_t, bp_t, out_t):
    import contextlib

    ctx = contextlib.ExitStack()
    with ctx:
        consts = ctx.enter_context(tc.tile_pool(name="consts", bufs=1))
        psum = ctx.enter_context(tc.tile_pool(name="psum", bufs=2, space="PSUM"))
        xin = ctx.enter_context(tc.tile_pool(name="xin", bufs=3))
        xcp = ctx.enter_context(tc.tile_pool(name="xcp", bufs=3))
        xtp = ctx.enter_context(tc.tile_pool(name="xtp", bufs=2))
        small = ctx.enter_context(tc.tile_pool(name="small", bufs=2))

        ident = consts.tile([P, P], CD)
        make_identity(nc, ident)

        # --- weights: load fp32, cast to CD on gpsimd ---
        wq_sb = consts.tile([P, NCH, C], CD)    # [p, c_chunk, qfeat]  = Wq[128c+p, :]
        wv_sb = consts.tile([P, NCH, C], CD)    # [p, c_chunk, vfeat]  = Wv[128c+p, :]
        wp_sb = consts.tile([P, NCH, C], CD)    # [p, c_chunk, ofeat]  = Wp[128c+p, :]
        wkT_sb = consts.tile([P, NCH, C], CD)   # [p, m_chunk, c]      = Wk[c, 128m+p]
        bp_sb = consts.tile([BL, C], F32)
        clsT_sb = consts.tile([P, NCH, BL], CD)  # per-head attention result, C-major

        with tc.tile_pool(name="wstage", bufs=2) as wstage:
            for w_ap, dst in (
                (wq_t[:, :].rearrange("(c p) f -> p c f", p=P), wq_sb),
                (wkv_t[:, C:].rearrange("(c p) f -> p c f", p=P), wv_sb),
                (wp_t[:, :].rearrange("(c p) f -> p c f", p=P), wp_sb),
            ):
                stg = wstage.tile([P, NCH, C], F32, tag="wstage")
                nc.sync.dma_start(out=stg, in_=w_ap)
                nc.gpsimd.tensor_copy(out=dst, in_=stg)
            # Wk needs a transpose: load + cast, then 36 PE transposes
            stg = wstage.tile([P, NCH, C], F32, tag="wstage")
            nc.sync.dma_start(
                out=stg, in_=wkv_t[:, :C].rearrange("(c p) f -> p c f", p=P)
            )
            wk_cd = wstage.tile([P, NCH, C], CD, tag="wkcd")
            nc.gpsimd.tensor_copy(out=wk_cd, in_=stg)
            for m in range(NCH):
                for c in range(NCH):
                    tp = psum.tile([P, P], CD, tag="tp")
                    nc.tensor.transpose(tp, wk_cd[:, c, m * P:(m + 1) * P], ident)
                    nc.vector.tensor_copy(out=wkT_sb[:, m, c * P:(c + 1) * P], in_=tp)

        nc.gpsimd.dma_start(
            out=bp_sb,
            in_=bass.AP(tensor=bp_t, offset=0, ap=[[0, BL], [1, C]]),
        )

        # ---------------- per batch ----------------
        for b in range(BL):
            # --- Q phase: qt (a.k.a. Q-tilde) [12, C] for this batch ---
            x0T_f = small.tile([P, NCH], F32, tag="x0Tf")
            nc.gpsimd.dma_start(
                out=x0T_f, in_=x_t[b, 0, :].rearrange("(c p) -> p c", p=P)
            )
            x0T = small.tile([P, NCH], CD, tag="x0T")
            nc.gpsimd.tensor_copy(out=x0T, in_=x0T_f)

            # q row [1, C] = x0 @ Wq  (fp32 accum in psum)
            qrow_ps = [psum.tile([1, HALF], F32, tag="sc", name=f"qrow_ps{i}") for i in range(2)]
            for half in range(2):
                for c in range(NCH):
                    nc.tensor.matmul(
                        qrow_ps[half],
                        lhsT=x0T[:, c:c + 1],
                        rhs=wq_sb[:, c, half * HALF:(half + 1) * HALF],
                        start=(c == 0),
                        stop=(c == NCH - 1),
                    )
            qrow_sb = small.tile([1, C], CD, tag="qrow")
            for half in range(2):
                nc.vector.tensor_copy(
                    out=qrow_sb[:, half * HALF:(half + 1) * HALF], in_=qrow_ps[half]
                )

            # qblock[p, m, h]: scaled q laid out block-diagonally per head pair
            qblock = small.tile([P, NCH, H], CD, tag="qblock")
            nc.vector.memset(qblock, 0.0)
            for m in range(NCH):
                qT_ps = psum.tile([P, 1], CD, tag="tp")
                nc.tensor.transpose(
                    qT_ps, qrow_sb[:, m * P:(m + 1) * P], ident[:1, :1]
                )
                nc.vector.tensor_scalar_mul(
                    qblock[0:D, m, 2 * m:2 * m + 1], qT_ps[0:D, :], SCALE
                )
                nc.vector.tensor_scalar_mul(
                    qblock[D:P, m, 2 * m + 1:2 * m + 2], qT_ps[D:P, :], SCALE
                )

            # qtrow [12, C] = blockdiag(q*scale)^T @ Wk^T  (accumulate over m)
            qt_ps = [psum.tile([H, HALF], F32, tag="sc", name=f"qt_ps{i}") for i in range(2)]
            for half in range(2):
                for m in range(NCH):
                    nc.tensor.matmul(
                        qt_ps[half],
                        lhsT=qblock[:, m, :],
                        rhs=wkT_sb[:, m, half * HALF:(half + 1) * HALF],
                        start=(m == 0),
                        stop=(m == NCH - 1),
                    )
            qtrow_sb = small.tile([H, C], CD, tag="qtrow")
            for half in range(2):
                nc.vector.tensor_copy(
                    out=qtrow_sb[:, half * HALF:(half + 1) * HALF], in_=qt_ps[half]
                )
            # transpose to qtT [p, c_chunk, h] for use as matmul stationary
            qtT_sb = small.tile([P, NCH, H], CD, tag="qtT")
            for c in range(NCH):
                tp = psum.tile([P, H], CD, tag="tp")
                nc.tensor.transpose(
                    tp, qtrow_sb[:, c * P:(c + 1) * P], ident[:H, :H]
                )
                nc.vector.tensor_copy(out=qtT_sb[:, c, :], in_=tp)

            # --- main streaming loop over token supertiles ---
            den_parts = small.tile([H, NST], F32, tag="den")
            u_ps = [psum.tile([H, HALF], F32, tag="u", name=f"u_ps{i}") for i in range(2)]

            for st in range(NST):
                xst = xin.tile([P, S, C], F32, tag="xin")
                nc.sync.dma_start(
                    out=xst,
                    in_=x_t[b, st * ST:(st + 1) * ST, :].rearrange(
                        "(p s) c -> p s c", s=S
                    ),
                )
                xc = xcp.tile([P, S, C], CD, tag="xcp")
                nc.gpsimd.tensor_copy(out=xc, in_=xst)

                # transpose x chunks: xT[:, c, s*128:(s+1)*128] = x[., s, c-chunk]^T
                xT = xtp.tile([P, NCH, ST], CD, tag="xtp")
                for c in range(NCH):
                    for s in range(S):
                        tp = psum.tile([P, P], CD, tag="tp")
                        nc.tensor.transpose(
                            tp, xc[:, s, c * P:(c + 1) * P], ident
                        )
                        nc.vector.tensor_copy(
                            out=xT[:, c, s * P:(s + 1) * P], in_=tp
                        )

                # scores [12, ST] accumulated over C chunks
                sc_ps = psum.tile([H, ST], F32, tag="sc")
                for c in range(NCH):
                    nc.tensor.matmul(
                        sc_ps,
                        lhsT=qtT_sb[:, c, :],
                        rhs=xT[:, c, :],
                        start=(c == 0),
                        stop=(c == NCH - 1),
                    )

                # e = exp(scores); accumulate denominator along free dim
                e_sb = small.tile([H, ST], CD, tag="e")
                nc.scalar.activation(
                    out=e_sb,
                    in_=sc_ps,
                    func=mybir.ActivationFunctionType.Exp,
                    accum_out=den_parts[:, st:st + 1],
                )

                # p^T per token group; weighted x accumulation into u
                for s in range(S):
                    pT_ps = psum.tile([P, H], CD, tag="tp")
                    nc.tensor.transpose(
                        pT_ps, e_sb[:, s * P:(s + 1) * P], ident[:H, :H]
                    )
                    pT_sb = small.tile([P, H], CD, tag="pT")
                    nc.vector.tensor_copy(out=pT_sb, in_=pT_ps)
                    for half in range(2):
                        nc.tensor.matmul(
                            u_ps[half],
                            lhsT=pT_sb,
                            rhs=xc[:, s, half * HALF:(half + 1) * HALF],
                            start=(st == 0 and s == 0),
                            stop=(st == NST - 1 and s == S - 1),
                        )

            # --- batch epilogue ---
            den = small.tile([H, 1], F32, tag="denf")
            nc.vector.reduce_sum(out=den, in_=den_parts, axis=mybir.AxisListType.X)
            rden = small.tile([H, 1], F32, tag="rden")
            nc.vector.reciprocal(out=rden, in_=den)

            ut_sb = small.tile([H, C], CD, tag="ut")
            for half in range(2):
                nc.vector.tensor_scalar_mul(
                    ut_sb[:, half * HALF:(half + 1) * HALF], u_ps[half], rden
                )
            utT_sb = small.tile([P, NCH, H], CD, tag="utT")
            for c in range(NCH):
                tp = psum.tile([P, H], CD, tag="tp")
                nc.tensor.transpose(tp, ut_sb[:, c * P:(c + 1) * P], ident[:H, :H])
                nc.vector.tensor_copy(out=utT_sb[:, c, :], in_=tp)

            # numfull [12, C] = ut @ Wv ; head h only needs cols [h*64,(h+1)*64)
            nf_ps = [psum.tile([H, HALF], F32, tag="u", name=f"nf_ps{i}") for i in range(2)]
            for half in range(2):
                for c in range(NCH):
                    nc.tensor.matmul(
                        nf_ps[half],
                        lhsT=utT_sb[:, c, :],
                        rhs=wv_sb[:, c, half * HALF:(half + 1) * HALF],
                        start=(c == 0),
                        stop=(c == NCH - 1),
                    )
            nf_sb = small.tile([H, C], CD, tag="nf")
            for half in range(2):
                nc.vector.tensor_copy(
                    out=nf_sb[:, half * HALF:(half + 1) * HALF], in_=nf_ps[half]
                )
            # extract block-diagonal -> clsT[:, c, b]
            for c in range(NCH):
                tp = psum.tile([P, H], CD, tag="tp")
                nc.tensor.transpose(tp, nf_sb[:, c * P:(c + 1) * P], ident[:H, :H])
                nc.vector.tensor_copy(
                    out=clsT_sb[0:D, c, b:b + 1], in_=tp[0:D, 2 * c:2 * c + 1]
                )
                nc.vector.tensor_copy(
                    out=clsT_sb[D:P, c, b:b + 1], in_=tp[D:P, 2 * c + 1:2 * c + 2]
                )

        # ---------------- output projection for all local batches ----------------
        o_ps = [psum.tile([BL, HALF], F32, tag="sc", name=f"o_ps{i}") for i in range(2)]
        for half in range(2):
            for c in range(NCH):
                nc.tensor.matmul(
                    o_ps[half],
                    lhsT=clsT_sb[:, c, :],
                    rhs=wp_sb[:, c, half * HALF:(half + 1) * HALF],
                    start=(c == 0),
                    stop=(c == NCH - 1),
                )
        o_sb = small.tile([BL, C], F32, tag="osb")
        for half in range(2):
            nc.vector.tensor_add(
                o_sb[:, half * HALF:(half + 1) * HALF],
                o_ps[half],
                bp_sb[:, half * HALF:(half + 1) * HALF],
            )
        nc.sync.dma_start(out=out_t[:, 0, :], in_=o_sb)


_NC_CACHE = None


def _get_nc():
    global _NC_CACHE
    if _NC_CACHE is None:
        _NC_CACHE = build()
    return _NC_CACHE


def kernel(x, Wq, Wkv, Wp, bp):
    nc = _get_nc()
    x = np.ascontiguousarray(x, dtype=np.float32)
    Wq = np.ascontiguousarray(Wq, dtype=np.float32)
    Wkv = np.ascontiguousarray(Wkv, dtype=np.float32)
    Wp = np.ascontiguousarray(Wp, dtype=np.float32)
    bp = np.ascontiguousarray(bp, dtype=np.float32)
    in_maps = [
        {
            "x": np.ascontiguousarray(x[i * BL:(i + 1) * BL]),
            "Wq": Wq,
            "Wkv": Wkv,
            "Wp": Wp,
            "bp": bp,
        }
        for i in range(NCORES)
    ]
    res = run_bass_kernel_spmd(nc, in_maps, core_ids=list(range(NCORES)))
    return np.concatenate([res.results[i]["out"] for i in range(NCORES)], axis=0)
